# revision 3
# baseline (speedup 1.0000x reference)
"""Trainium2 Bass kernel for nn_DirectionalAlignment.

Computation (per batch b, plane p), with x: (B, P, S, D), w: (P, S, S):
    scores = x @ x.T / sqrt(D)            # (S, S)
    red    = sum(scores * w[p], axis=-1)  # (S, 1)
    y      = x + red
    out    = BatchNorm2d(y)               # per-plane stats over (B, S, D)

Rewritten to avoid materializing scores:
    u   = (w[p]/sqrt(D)) @ x              # (S, D) matmul, f16 on the PE
    red = rowsum(x * u)                   # DVE scalar_tensor_tensor accum
    y   = x + red                         # per-b tensor_scalar (4x f16 mode),
                                          #   accum gives per-col sum(y) free
    Sy2 = sum(y^2)                        # ACT Square, one BIG op per m-chunk
    out = c0*y + c1                       # uniform scalars -> one BIG op per
                                          #   m-chunk (c0=gamma*istd,
                                          #   c1=beta-mean*c0)

All I/O is f16 (tolerance is 2e-2; f16 keeps l2 err ~3e-4) which halves
DMA traffic vs f32: per core 8MB x in + 8MB out + 1MB w -> ~49us at
360GB/s.  Engine budget per plane: DVE ~8.6us (16 rowsum STT from PSUM
at 1x + yadd share + pass2 share + tinies), ACT ~8.5us (yadd share +
2 big Squares + pass2 share), PE ~3.4us, Pool: all_reduce only.

Sharding: planes (P=64) split across 8 cores, 8 planes each — BN stats are
per-plane so no collectives are needed.  The host pre-transposes each
core's x slice so every SBUF tile load is one contiguous DMA:
    xh[r, p, tc, b, d] = x[b, plane, tc*128 + r, d]   (128, PPC*2*B*D) f16
Weights are pre-transposed and pre-scaled by 1/sqrt(D):
    wh[r, p, tc, s] = w[plane, s, tc*128 + r] / sqrt(D)  f16
The output leaves the device f16 in the same transposed layout; the host
inverse-transposes and upcasts to f32 when gathering.

Software pipeline: the stats-finalize + pass2 + store of plane p are
emitted after the heavy ops of plane p+1, so the long per-plane stat
dependency chain never head-of-line blocks the DVE/ACT streams.
"""

import numpy as np
from contextlib import ExitStack

B, P, S, D = 8, 64, 256, 256
N_CORES = 8
PPC = P // N_CORES  # planes per core
BN_EPS = 1e-5
NTOT = B * S * D  # elements per plane for BN stats
PFREE = 2 * B * D  # 4096 per-partition elements per plane
XFREE = PPC * PFREE  # 32768 per-partition elements per core

_CACHE = {}


def _build_nc(reps=1, yadd_dve=14, p5_act=(0,), xt_bufs=4, yt_bufs=3,
              ot_bufs=3):
    import concourse.tile as tile
    from concourse import bacc, mybir, bass_isa

    F32 = mybir.dt.float32
    F16 = mybir.dt.float16
    Alu = mybir.AluOpType
    Act = mybir.ActivationFunctionType

    nc = bacc.Bacc("TRN2", target_bir_lowering=False, debug=False,
                   enable_asserts=False)
    x_d = nc.dram_tensor("x", [128, XFREE], F16, kind="ExternalInput").ap()
    wt_d = nc.dram_tensor("wt", [128, PPC * 2 * S], F16,
                          kind="ExternalInput").ap()
    g_d = nc.dram_tensor("gamma", [1, PPC], F32, kind="ExternalInput").ap()
    be_d = nc.dram_tensor("beta", [1, PPC], F32, kind="ExternalInput").ap()
    o_d = nc.dram_tensor("out", [128, XFREE], F16, kind="ExternalOutput").ap()

    with tile.TileContext(nc) as tc, ExitStack() as ctx:
        xpool = ctx.enter_context(tc.tile_pool(name="xp", bufs=xt_bufs))
        ypool = ctx.enter_context(tc.tile_pool(name="yp", bufs=yt_bufs))
        opool = ctx.enter_context(tc.tile_pool(name="op", bufs=ot_bufs))
        wpool = ctx.enter_context(tc.tile_pool(name="wp", bufs=1))
        upool = ctx.enter_context(tc.tile_pool(name="u", bufs=2, space="PSUM"))
        prodpool = ctx.enter_context(tc.tile_pool(name="prod", bufs=6))
        sqpool = ctx.enter_context(tc.tile_pool(name="sqs", bufs=3))
        accpool = ctx.enter_context(tc.tile_pool(name="acc", bufs=3))
        tiny = ctx.enter_context(tc.tile_pool(name="tiny", bufs=36))
        const = ctx.enter_context(tc.tile_pool(name="const", bufs=1))

        # gamma/beta broadcast to all partitions (once, outside the loop)
        g1 = const.tile([1, PPC], F32)
        b1 = const.tile([1, PPC], F32)
        gb = const.tile([128, PPC], F32)
        bb = const.tile([128, PPC], F32)
        nc.sync.dma_start(g1[:], g_d[:])
        nc.sync.dma_start(b1[:], be_d[:])
        nc.gpsimd.partition_broadcast(gb[:], g1[:], channels=128)
        nc.gpsimd.partition_broadcast(bb[:], b1[:], channels=128)

        def heavy(p, state):
            """DMA + matmuls + rowsum + y-add + Sy2 for plane p, ending
            with the partition all_reduce of the plane's stats."""
            xt = xpool.tile([128, 2, B, D], F16, name=f"xt{p}", tag="xt")
            nc.sync.dma_start(
                xt[:].rearrange("r t b d -> r (t b d)"),
                x_d[:, p * PFREE:(p + 1) * PFREE])
            wt = state["wall"][:, p]  # (128, 2, S)

            yt = ypool.tile([128, 2, B, D], F16, name=f"yt{p}", tag="yt")
            red = accpool.tile([128, 16], F32, name=f"red{p}", tag="red")
            sy = accpool.tile([128, 16], F32, name=f"sy{p}", tag="sy")
            q2 = accpool.tile([128, 2], F32, name=f"q2{p}", tag="q2")

            for m in range(2):  # output row-chunk of u (s axis)
                u_ps = upool.tile([128, B, D], F32)
                for k in range(2):  # contraction chunk (t axis)
                    for j in range(4):  # pairs of batches
                        nc.tensor.matmul(
                            u_ps[:, 2 * j:2 * j + 2, :],
                            wt[:, k, m * 128:(m + 1) * 128],
                            xt[:, k, 2 * j:2 * j + 2, :],
                            start=(k == 0), stop=(k == 1))

                for b in range(B):
                    c = m * B + b
                    # red[s,(m,b)] = sum_d x*u
                    prod = prodpool.tile([128, D], F16, name=f"pr{p}_{c}",
                                         tag="prod")
                    nc.vector.scalar_tensor_tensor(
                        out=prod[:], in0=xt[:, m, b], scalar=1.0,
                        in1=u_ps[:, b], op0=Alu.mult, op1=Alu.mult,
                        accum_out=red[:, c:c + 1])
                for b in range(B):
                    c = m * B + b
                    # y = x + red; accum -> sy[(m,b)] = sum_d y
                    if c < yadd_dve:
                        nc.vector.tensor_scalar(
                            out=yt[:, m, b], in0=xt[:, m, b],
                            scalar1=red[:, c:c + 1], scalar2=0.0,
                            op0=Alu.add, op1=Alu.add,
                            accum_out=sy[:, c:c + 1])
                    else:
                        nc.scalar.activation(
                            yt[:, m, b], xt[:, m, b], Act.Identity,
                            bias=red[:, c:c + 1], scale=1.0,
                            accum_out=sy[:, c:c + 1])
                # Sy2 for this m-chunk: one BIG ACT Square with accum
                sq = sqpool.tile([128, B, D], F16, name=f"sq{p}_{m}",
                                 tag="sq")
                nc.scalar.activation(sq[:], yt[:, m], Act.Square,
                                     accum_out=q2[:, m:m + 1])

            # per-partition totals -> all_reduce over partitions
            st = tiny.tile([128, 2], F32, name=f"st{p}", tag="st")
            nc.vector.tensor_reduce(st[:, 0:1], sy[:],
                                    axis=mybir.AxisListType.X, op=Alu.add)
            nc.vector.tensor_reduce(st[:, 1:2], q2[:],
                                    axis=mybir.AxisListType.X, op=Alu.add)
            tot = tiny.tile([128, 2], F32, name=f"tot{p}", tag="tot")
            nc.gpsimd.partition_all_reduce(tot[:], st[:], channels=128,
                                           reduce_op=bass_isa.ReduceOp.add)
            state[p] = (yt, tot)

        def tail(p, state):
            """Finalize BN stats, pass2, store for plane p."""
            yt, tot = state.pop(p)
            mm = tiny.tile([128, 2], F32, name=f"mm{p}", tag="mm")
            nc.vector.tensor_scalar(out=mm[:], in0=tot[:],
                                    scalar1=1.0 / NTOT, scalar2=0.0,
                                    op0=Alu.mult, op1=Alu.add)
            m2 = tiny.tile([128, 1], F32, name=f"m2{p}", tag="m2")
            nc.vector.tensor_tensor(m2[:], mm[:, 0:1], mm[:, 0:1],
                                    op=Alu.mult)
            vps = tiny.tile([128, 1], F32, name=f"vps{p}", tag="vps")
            nc.vector.scalar_tensor_tensor(
                out=vps[:], in0=mm[:, 1:2], scalar=BN_EPS, in1=m2[:],
                op0=Alu.add, op1=Alu.subtract)
            rcp = tiny.tile([128, 1], F32, name=f"rcp{p}", tag="rcp")
            nc.vector.reciprocal(rcp[:], vps[:])
            istd = tiny.tile([128, 1], F32, name=f"istd{p}", tag="istd")
            nc.scalar.activation(istd[:], rcp[:], Act.Sqrt)
            c0 = tiny.tile([128, 1], F32, name=f"c0{p}", tag="c0")
            nc.vector.tensor_tensor(c0[:], gb[:, p:p + 1], istd[:],
                                    op=Alu.mult)
            nmc = tiny.tile([128, 1], F32, name=f"nmc{p}", tag="nmc")
            nc.vector.scalar_tensor_tensor(
                out=nmc[:], in0=mm[:, 0:1], scalar=-1.0, in1=c0[:],
                op0=Alu.mult, op1=Alu.mult)  # -mean*c0
            c1 = tiny.tile([128, 1], F32, name=f"c1{p}", tag="c1")
            nc.vector.tensor_tensor(c1[:], bb[:, p:p + 1], nmc[:],
                                    op=Alu.add)

            # pass2: out = c0*y + c1, one big op per m-chunk
            ot = opool.tile([128, 2, B, D], F16, name=f"ot{p}", tag="ot")
            for m in range(2):
                if m in p5_act:
                    nc.scalar.activation(ot[:, m], yt[:, m], Act.Identity,
                                         bias=c1[:], scale=c0[:])
                else:
                    nc.vector.tensor_scalar(
                        out=ot[:, m], in0=yt[:, m], scalar1=c0[:],
                        scalar2=c1[:], op0=Alu.mult, op1=Alu.add)
            nc.sync.dma_start(
                o_d[:, p * PFREE:(p + 1) * PFREE],
                ot[:].rearrange("r t b d -> r (t b d)"))

        def body():
            state = {}
            wall = wpool.tile([128, PPC, 2, S], F16)
            nc.sync.dma_start(
                wall[:].rearrange("r p t s -> r (p t s)"), wt_d[:])
            state["wall"] = wall
            for p in range(PPC):
                heavy(p, state)
                if p > 0:
                    tail(p - 1, state)
            tail(PPC - 1, state)

        if reps == 1:
            body()
        else:
            with tc.For_i(0, reps, 1):
                body()

    nc.compile()
    return nc


def _get_nc(**kw):
    key = tuple(sorted(kw.items()))
    if key not in _CACHE:
        _CACHE[key] = _build_nc(**kw)
    return _CACHE[key]


def _make_in_maps(x, weights, gamma, beta):
    inv = np.float32(1.0 / np.sqrt(D))
    # wh[r, p_all, tc, s] = w[p_all, s, tc*128+r] / sqrt(D)
    wh = (weights.reshape(P, S, 2, 128).transpose(3, 0, 2, 1)
          * inv).astype(np.float16)
    in_maps = []
    for c in range(N_CORES):
        sl = slice(c * PPC, (c + 1) * PPC)
        # xh[r, p, tc, b, d] = x[b, plane, tc*128+r, d]
        xc = x[:, sl].reshape(B, PPC, 2, 128, D)
        xh = xc.transpose(3, 1, 2, 0, 4).astype(np.float16).reshape(128, XFREE)
        in_maps.append({
            "x": xh,
            "wt": np.ascontiguousarray(wh[:, sl]).reshape(128, PPC * 2 * S),
            "gamma": np.ascontiguousarray(gamma[sl]).reshape(1, PPC),
            "beta": np.ascontiguousarray(beta[sl]).reshape(1, PPC),
        })
    return in_maps


def _gather_out(results):
    # invert: oh (128, PPC, 2, B, D) f16 -> (B, PPC, S, D) f32 per core
    outs = []
    for c in range(N_CORES):
        oh = results[c]["out"].reshape(128, PPC, 2, B, D)
        oc = oh.transpose(3, 1, 2, 0, 4).astype(np.float32).reshape(
            B, PPC, S, D)
        outs.append(oc)
    return np.ascontiguousarray(np.concatenate(outs, axis=1))


def kernel(x, weights, gamma, beta):
    from concourse.bass_utils import run_bass_kernel_spmd
    x = np.asarray(x, dtype=np.float32)
    weights = np.asarray(weights, dtype=np.float32)
    gamma = np.asarray(gamma, dtype=np.float32)
    beta = np.asarray(beta, dtype=np.float32)

    nc = _get_nc()
    in_maps = _make_in_maps(x, weights, gamma, beta)
    res = run_bass_kernel_spmd(nc, in_maps, core_ids=list(range(N_CORES)))
    return _gather_out(res.results)


# revision 15
# speedup vs baseline: 2.4902x; 2.4902x over previous
"""Trainium2 Bass kernel for nn_DirectionalAlignment.

Computation (per batch b, plane p), with x: (B, P, S, D), w: (P, S, S):
    scores = x @ x.T / sqrt(D)            # (S, S)
    red    = sum(scores * w[p], axis=-1)  # (S, 1)
    y      = x + red
    out    = BatchNorm2d(y)               # per-plane stats over (B, S, D)

Rewritten to avoid materializing scores:
    u   = (w[p]/sqrt(D)) @ x              # (S, D) matmul, f16 on the PE
    red = rowsum(x * u)                   # DVE scalar_tensor_tensor accum
    y   = x + red                         # per-b tensor_scalar (4x f16 mode),
                                          #   accum gives per-col sum(y) free
    Sy2 = sum(y^2)                        # ACT Square, one BIG op per m-chunk
    out = c0*y + c1                       # uniform scalars -> one BIG op per
                                          #   m-chunk (c0=gamma*istd,
                                          #   c1=beta-mean*c0)

All I/O is f16 (tolerance is 2e-2; f16 keeps l2 err ~3e-4) which halves
DMA traffic vs f32: per core 8MB x in + 8MB out + 1MB w -> ~49us at
360GB/s.  Engine budget per plane: DVE ~8.6us (16 rowsum STT from PSUM
at 1x + yadd share + pass2 share + tinies), ACT ~8.5us (yadd share +
2 big Squares + pass2 share), PE ~3.4us, Pool: all_reduce only.

Sharding: planes (P=64) split across 8 cores, 8 planes each — BN stats are
per-plane so no collectives are needed.  The host pre-transposes each
core's x slice so every SBUF tile load is one contiguous DMA:
    xh[r, p, tc, b, d] = x[b, plane, tc*128 + r, d]   (128, PPC*2*B*D) f16
Weights are pre-transposed and pre-scaled by 1/sqrt(D):
    wh[r, p, tc, s] = w[plane, s, tc*128 + r] / sqrt(D)  f16
The output leaves the device f16 in the same transposed layout; the host
inverse-transposes and upcasts to f32 when gathering.

Software pipeline: the stats-finalize + pass2 + store of plane p are
emitted after the heavy ops of plane p+1, so the long per-plane stat
dependency chain never head-of-line blocks the DVE/ACT streams.
"""

import numpy as np
from contextlib import ExitStack

B, P, S, D = 8, 64, 256, 256
N_CORES = 8
PPC = P // N_CORES  # planes per core
BN_EPS = 1e-5
NTOT = B * S * D  # elements per plane for BN stats
PFREE = 2 * B * D  # 4096 per-partition elements per plane
XFREE = PPC * PFREE  # 32768 per-partition elements per core

_CACHE = {}


def _build_nc(reps=1, n_pool_yadd=0, p5_act=(0, 1), chunk=1, xt_bufs=4,
              yt_bufs=3, ot_bufs=3, x_eng="sync", o_eng="sync"):
    import concourse.tile as tile
    from concourse import bacc, mybir, bass_isa

    F32 = mybir.dt.float32
    F16 = mybir.dt.float16
    Alu = mybir.AluOpType
    Act = mybir.ActivationFunctionType

    nc = bacc.Bacc("TRN2", target_bir_lowering=False, debug=False,
                   enable_asserts=False)
    x_d = nc.dram_tensor("x", [128, XFREE], F16, kind="ExternalInput").ap()
    wt_d = nc.dram_tensor("wt", [128, PPC * 2 * S], F16,
                          kind="ExternalInput").ap()
    g_d = nc.dram_tensor("gamma", [1, PPC], F32, kind="ExternalInput").ap()
    be_d = nc.dram_tensor("beta", [1, PPC], F32, kind="ExternalInput").ap()
    o_d = nc.dram_tensor("out", [128, XFREE], F16, kind="ExternalOutput").ap()

    with tile.TileContext(nc) as tc, ExitStack() as ctx:
        xpool = ctx.enter_context(tc.tile_pool(name="xp", bufs=xt_bufs))
        ypool = ctx.enter_context(tc.tile_pool(name="yp", bufs=yt_bufs))
        opool = ctx.enter_context(tc.tile_pool(name="op", bufs=ot_bufs))
        wpool = ctx.enter_context(tc.tile_pool(name="wp", bufs=1))
        upool = ctx.enter_context(tc.tile_pool(name="u", bufs=2, space="PSUM"))
        prodpool = ctx.enter_context(tc.tile_pool(name="prod", bufs=6))
        sqpool = ctx.enter_context(tc.tile_pool(name="sqs", bufs=3))
        accpool = ctx.enter_context(tc.tile_pool(name="acc", bufs=3))
        tiny = ctx.enter_context(tc.tile_pool(name="tiny", bufs=36))
        const = ctx.enter_context(tc.tile_pool(name="const", bufs=1))

        # gamma/beta broadcast to all partitions (once, outside the loop)
        g1 = const.tile([1, PPC], F32)
        b1 = const.tile([1, PPC], F32)
        gb = const.tile([128, PPC], F32)
        bb = const.tile([128, PPC], F32)
        nc.sync.dma_start(g1[:], g_d[:])
        nc.sync.dma_start(b1[:], be_d[:])
        nc.gpsimd.partition_broadcast(gb[:], g1[:], channels=128)
        nc.gpsimd.partition_broadcast(bb[:], b1[:], channels=128)

        def heavy(p, state):
            """DMA + matmuls + rowsum + y-add + Sy2 for plane p, ending
            with the partition all_reduce of the plane's stats."""
            if p % chunk == 0:
                xc = xpool.tile([128, chunk, 2, B, D], F16,
                                name=f"xc{p // chunk}", tag="xc")
                getattr(nc, x_eng).dma_start(
                    xc[:].rearrange("r c t b d -> r (c t b d)"),
                    x_d[:, p * PFREE:(p + chunk) * PFREE])
                state["xc"] = xc
            xt = state["xc"][:, p % chunk]  # (128, 2, B, D)
            wt = state["wall"][:, p]  # (128, 2, S)

            yt = ypool.tile([128, 2, B, D], F16, name=f"yt{p}", tag="yt")
            red = accpool.tile([128, 2, B], F32, name=f"red{p}", tag="red")
            sy = accpool.tile([128, 2], F32, name=f"sy{p}", tag="sy")
            q2 = accpool.tile([128, 2], F32, name=f"q2{p}", tag="q2")

            for m in range(2):  # output row-chunk of u (s axis)
                u_ps = upool.tile([128, B, D], F32)
                for k in range(2):  # contraction chunk (t axis)
                    for j in range(4):  # pairs of batches
                        nc.tensor.matmul(
                            u_ps[:, 2 * j:2 * j + 2, :],
                            wt[:, k, m * 128:(m + 1) * 128],
                            xt[:, k, 2 * j:2 * j + 2, :],
                            start=(k == 0), stop=(k == 1))

                for b in range(B):
                    # red[s,(m,b)] = sum_d x*u
                    prod = prodpool.tile([128, D], F16, name=f"pr{p}_{m}{b}",
                                         tag="prod")
                    nc.vector.scalar_tensor_tensor(
                        out=prod[:], in0=xt[:, m, b], scalar=1.0,
                        in1=u_ps[:, b], op0=Alu.mult, op1=Alu.mult,
                        accum_out=red[:, m, b:b + 1])
                # y = x + red broadcast over d: one big DVE op per m-chunk,
                # accum gives sum(y) for free
                red_b = red[:, m].unsqueeze(2).broadcast_to((128, B, D))
                if m < n_pool_yadd:
                    nc.gpsimd.tensor_tensor(yt[:, m], xt[:, m], red_b,
                                            op=Alu.add)
                    sq0 = sqpool.tile([128, B, D], F16, name=f"si{p}_{m}",
                                      tag="sq")
                    nc.scalar.activation(sq0[:], yt[:, m], Act.Identity,
                                         accum_out=sy[:, m:m + 1])
                else:
                    nc.vector.scalar_tensor_tensor(
                        out=yt[:, m], in0=xt[:, m], scalar=0.0, in1=red_b,
                        op0=Alu.add, op1=Alu.add, accum_out=sy[:, m:m + 1])
                # Sy2 for this m-chunk: BIG ACT Square with accum
                sq = sqpool.tile([128, B, D], F16, name=f"sq{p}_{m}",
                                 tag="sq")
                nc.scalar.activation(sq[:], yt[:, m], Act.Square,
                                     accum_out=q2[:, m:m + 1])

            # per-partition totals -> all_reduce over partitions
            st = tiny.tile([128, 2], F32, name=f"st{p}", tag="st")
            nc.vector.tensor_reduce(st[:, 0:1], sy[:],
                                    axis=mybir.AxisListType.X, op=Alu.add)
            nc.vector.tensor_reduce(st[:, 1:2], q2[:],
                                    axis=mybir.AxisListType.X, op=Alu.add)
            tot = tiny.tile([128, 2], F32, name=f"tot{p}", tag="tot")
            nc.gpsimd.partition_all_reduce(tot[:], st[:], channels=128,
                                           reduce_op=bass_isa.ReduceOp.add)
            state[p] = (yt, tot)

        def tail(p, state):
            """Finalize BN stats, pass2, store for plane p."""
            yt, tot = state.pop(p)
            mm = tiny.tile([128, 2], F32, name=f"mm{p}", tag="mm")
            nc.vector.tensor_scalar(out=mm[:], in0=tot[:],
                                    scalar1=1.0 / NTOT, scalar2=0.0,
                                    op0=Alu.mult, op1=Alu.add)
            m2 = tiny.tile([128, 1], F32, name=f"m2{p}", tag="m2")
            nc.vector.tensor_tensor(m2[:], mm[:, 0:1], mm[:, 0:1],
                                    op=Alu.mult)
            vps = tiny.tile([128, 1], F32, name=f"vps{p}", tag="vps")
            nc.vector.scalar_tensor_tensor(
                out=vps[:], in0=mm[:, 1:2], scalar=BN_EPS, in1=m2[:],
                op0=Alu.add, op1=Alu.subtract)
            rcp = tiny.tile([128, 1], F32, name=f"rcp{p}", tag="rcp")
            nc.vector.reciprocal(rcp[:], vps[:])
            istd = tiny.tile([128, 1], F32, name=f"istd{p}", tag="istd")
            nc.scalar.activation(istd[:], rcp[:], Act.Sqrt)
            c0 = tiny.tile([128, 1], F32, name=f"c0{p}", tag="c0")
            nc.vector.tensor_tensor(c0[:], gb[:, p:p + 1], istd[:],
                                    op=Alu.mult)
            nmc = tiny.tile([128, 1], F32, name=f"nmc{p}", tag="nmc")
            nc.vector.scalar_tensor_tensor(
                out=nmc[:], in0=mm[:, 0:1], scalar=-1.0, in1=c0[:],
                op0=Alu.mult, op1=Alu.mult)  # -mean*c0
            c1 = tiny.tile([128, 1], F32, name=f"c1{p}", tag="c1")
            nc.vector.tensor_tensor(c1[:], bb[:, p:p + 1], nmc[:],
                                    op=Alu.add)

            # pass2: out = c0*y + c1, one big op per m-chunk
            if p % chunk == 0:
                state["otc"] = opool.tile([128, chunk, 2, B, D], F16,
                                          name=f"otc{p // chunk}", tag="otc")
            ot = state["otc"][:, p % chunk]
            for m in range(2):
                if m in p5_act:
                    nc.scalar.activation(ot[:, m], yt[:, m], Act.Identity,
                                         bias=c1[:], scale=c0[:])
                else:
                    nc.vector.tensor_scalar(
                        out=ot[:, m], in0=yt[:, m], scalar1=c0[:],
                        scalar2=c1[:], op0=Alu.mult, op1=Alu.add)
            if p % chunk == chunk - 1:
                getattr(nc, o_eng).dma_start(
                    o_d[:, (p - chunk + 1) * PFREE:(p + 1) * PFREE],
                    state["otc"][:].rearrange("r c t b d -> r (c t b d)"))

        def body():
            state = {}
            wall = wpool.tile([128, PPC, 2, S], F16)
            nc.sync.dma_start(
                wall[:].rearrange("r p t s -> r (p t s)"), wt_d[:])
            state["wall"] = wall
            for p in range(PPC):
                heavy(p, state)
                if p > 0:
                    tail(p - 1, state)
            tail(PPC - 1, state)

        if reps == 1:
            body()
        else:
            with tc.For_i(0, reps, 1):
                body()

    nc.compile()
    return nc


def _get_nc(**kw):
    key = tuple(sorted(kw.items()))
    if key not in _CACHE:
        _CACHE[key] = _build_nc(**kw)
    return _CACHE[key]


def _make_in_maps(x, weights, gamma, beta):
    inv = np.float32(1.0 / np.sqrt(D))
    # wh[r, p_all, tc, s] = w[p_all, s, tc*128+r] / sqrt(D)
    wh = (weights.reshape(P, S, 2, 128).transpose(3, 0, 2, 1)
          * inv).astype(np.float16)
    in_maps = []
    for c in range(N_CORES):
        sl = slice(c * PPC, (c + 1) * PPC)
        # xh[r, p, tc, b, d] = x[b, plane, tc*128+r, d]
        xc = x[:, sl].reshape(B, PPC, 2, 128, D)
        xh = xc.transpose(3, 1, 2, 0, 4).astype(np.float16).reshape(128, XFREE)
        in_maps.append({
            "x": xh,
            "wt": np.ascontiguousarray(wh[:, sl]).reshape(128, PPC * 2 * S),
            "gamma": np.ascontiguousarray(gamma[sl]).reshape(1, PPC),
            "beta": np.ascontiguousarray(beta[sl]).reshape(1, PPC),
        })
    return in_maps


def _gather_out(results):
    # invert: oh (128, PPC, 2, B, D) f16 -> (B, PPC, S, D) f32 per core
    outs = []
    for c in range(N_CORES):
        oh = results[c]["out"].reshape(128, PPC, 2, B, D)
        oc = oh.transpose(3, 1, 2, 0, 4).astype(np.float32).reshape(
            B, PPC, S, D)
        outs.append(oc)
    return np.ascontiguousarray(np.concatenate(outs, axis=1))


def kernel(x, weights, gamma, beta):
    from concourse.bass_utils import run_bass_kernel_spmd
    x = np.asarray(x, dtype=np.float32)
    weights = np.asarray(weights, dtype=np.float32)
    gamma = np.asarray(gamma, dtype=np.float32)
    beta = np.asarray(beta, dtype=np.float32)

    nc = _get_nc()
    in_maps = _make_in_maps(x, weights, gamma, beta)
    res = run_bass_kernel_spmd(nc, in_maps, core_ids=list(range(N_CORES)))
    return _gather_out(res.results)
